# revision 1
# baseline (speedup 1.0000x reference)
"""LIF neuron scan kernel v3 for Trainium2 (8 NeuronCores).

Reference recurrence per timestep t (elementwise over B x N):
    u = (x_t - m)*sig + m ; s = (u >= th) ; m = (1-s)*u
with sig = sigmoid(tau_x) in (0,1), th > 0 per neuron.

v3 reformulation ("u-hat space"): track uh = u / th. Then
    uh' = cm * uh * (1 - s) + x * (sig/th),   s = (uh >= 1),  cm = 1 - sig.
The threshold compare becomes a compare against the IMMEDIATE 1.0 and fuses
with the reset complement and decay into ONE scalar_tensor_tensor:
    w  = (uh < 1) * cm        # stt: (in0 is_lt 1.0) mult cm-tile
    p  = uh * w               # tensor_tensor mult
    uh'= p + xhs'             # tensor_tensor add,  xhs = x * (sig/th)
=> THREE serial DVE ops per timestep (the baseline had five).

Engine assignment (measured on this part: GpSimd activity degrades DVE ~2.6x
even on disjoint tiles, so GpSimd is not used at all; ACT runs interference-
free next to DVE):
    DVE : per-t chain (stt, mult, add) + two amortized [128, 4096] block ops
          per 8-timestep block: xhs = x*sigth, and mems16 = p*(th/cm) written
          directly as bf16 (m = p*th/cm is the exact reference m = u*(1-s)).
    ACT : spikes16 = Exp(-1e30 * w) -> bf16. w == 0 exactly iff spike, else
          w = cm in [0.1, 0.9]: exp gives exactly 1.0 / 0.0 (underflow).
    sync: all DMA issue. Outputs travel as bf16 (halves output bytes; spikes
          bf16 is lossless 0/1, mems bf16 is ~0.2% rounding, far inside the
          2e-2 gate); host widens to f32.

Sharding: data-parallel over batch B across 8 cores (4 batches/core),
constants replicated; cores fully independent (recurrence is only over T).
Per-core layout: [128, 512] per timestep, partition p = b_local*32 + sub,
free = n_low, neuron n = sub*512 + n_low.

Raw Bass: standalone wait_ge + at most one then_inc per instruction; DMA
completion sems only waited at full-count values (single sync DMA queue
keeps completion increments ordered).
"""

import sys

if "/opt/trn_rl_repo" not in sys.path:
    sys.path.insert(0, "/opt/trn_rl_repo")

import numpy as np

import concourse.bass as bass
import concourse.mybir as mybir
from concourse.bass_utils import run_bass_kernel_spmd

B, T, N = 32, 64, 16384
NCORES = 8
BL = B // NCORES
SUB = 32
NL = N // SUB  # 512
P = BL * SUB  # 128
TBLK = 8
NBLK = T // TBLK
BW = TBLK * NL  # 4096
F32 = mybir.dt.float32
BF16 = mybir.dt.bfloat16
ALU = mybir.AluOpType
AF = mybir.ActivationFunctionType

_CACHE: dict = {}


def _build_nc() -> bass.Bass:
    nc = bass.Bass()
    x = nc.dram_tensor("x", [BL, T, N], F32, kind="ExternalInput")
    cm_d = nc.dram_tensor("cm", [N], F32, kind="ExternalInput")
    sigth_d = nc.dram_tensor("sigth", [N], F32, kind="ExternalInput")
    thcm_d = nc.dram_tensor("thcm", [N], F32, kind="ExternalInput")
    spikes16 = nc.dram_tensor("spikes16", [BL, T, N], BF16, kind="ExternalOutput")
    mems16 = nc.dram_tensor("mems16", [BL, T, N], BF16, kind="ExternalOutput")

    cm_2d = cm_d.rearrange("(s n) -> s n", n=NL)
    sigth_2d = sigth_d.rearrange("(s n) -> s n", n=NL)
    thcm_2d = thcm_d.rearrange("(s n) -> s n", n=NL)

    def x_src(b, k):
        return x[b, k * TBLK : (k + 1) * TBLK, :].rearrange(
            "t (s n) -> s t n", n=NL
        )

    def out_dst(dram, b, k):
        return dram[b, k * TBLK : (k + 1) * TBLK, :].rearrange(
            "t (s n) -> s t n", n=NL
        )

    def blkv(tile, b):
        return tile[b * SUB : (b + 1) * SUB, :].rearrange("p (t n) -> p t n", n=NL)

    import contextlib
    with contextlib.ExitStack() as _stack:
        cm_t = _stack.enter_context(nc.sbuf_tensor([P, NL], F32))
        sigth_t = _stack.enter_context(nc.sbuf_tensor([P, NL], F32))
        thcm_t = _stack.enter_context(nc.sbuf_tensor([P, NL], F32))
        sigthb_t = _stack.enter_context(nc.sbuf_tensor([P, BW], F32))
        thcmb_t = _stack.enter_context(nc.sbuf_tensor([P, BW], F32))
        uh_t = _stack.enter_context(nc.sbuf_tensor([P, 2 * NL], F32))
        xb_all = _stack.enter_context(nc.sbuf_tensor([P, 3 * BW], F32))
        xs_t = _stack.enter_context(nc.sbuf_tensor([P, BW], F32))
        w_all = _stack.enter_context(nc.sbuf_tensor([P, BW], F32))
        p_all = _stack.enter_context(nc.sbuf_tensor([P, 2 * BW], F32))
        m16_all = _stack.enter_context(nc.sbuf_tensor([P, 2 * BW], BF16))
        s16_all = _stack.enter_context(nc.sbuf_tensor([P, 2 * BW], BF16))
        c_sem = _stack.enter_context(nc.semaphore("c_sem"))
        rep_sem = _stack.enter_context(nc.semaphore("rep_sem"))
        x_sem = _stack.enter_context(nc.semaphore("x_sem"))
        x0_sem = _stack.enter_context(nc.semaphore("x0_sem"))
        xsd_sem = _stack.enter_context(nc.semaphore("xsd_sem"))
        w_sem = _stack.enter_context(nc.semaphore("w_sem"))
        m16_sem = _stack.enter_context(nc.semaphore("m16_sem"))
        spk_sem = _stack.enter_context(nc.semaphore("spk_sem"))
        mo_sem = _stack.enter_context(nc.semaphore("mo_sem"))
        so_sem = _stack.enter_context(nc.semaphore("so_sem"))
        block = _stack.enter_context(nc.Block())
        xb_r = [xb_all[:, r * BW : (r + 1) * BW] for r in range(3)]
        m16_r = [m16_all[:, r * BW : (r + 1) * BW] for r in range(2)]
        s16_r = [s16_all[:, r * BW : (r + 1) * BW] for r in range(2)]

        def wsl(tl):
            return w_all[:, tl * NL : (tl + 1) * NL]

        def psl(k, tl):
            r = k % 2
            return p_all[:, (r * TBLK + tl) * NL : (r * TBLK + tl + 1) * NL]

        def pblk(k):
            r = k % 2
            return p_all[:, r * BW : (r + 1) * BW]

        def uhsl(t):
            r = t % 2
            return uh_t[:, r * NL : (r + 1) * NL]

        @block.sync
        def _(sync):
            for src, dst in ((sigth_2d, sigth_t), (cm_2d, cm_t)):
                for b in range(BL):
                    sync.dma_start(
                        out=dst[b * SUB : (b + 1) * SUB, :], in_=src
                    ).then_inc(c_sem, 16)
            for b in range(BL):
                sync.dma_start(
                    out=thcm_t[b * SUB : (b + 1) * SUB, :], in_=thcm_2d
                ).then_inc(c_sem, 16)
            for b in range(BL):
                sync.dma_start(out=blkv(xb_r[1], b), in_=x_src(b, 1)).then_inc(
                    x_sem, 16
                )
            for k in range(NBLK):
                kf = k + 2  # fetch 2 blocks ahead
                if kf < NBLK:
                    sync.wait_ge(xsd_sem, max(kf - 2, 0))  # xb slot WAR
                    sync.wait_ge(x_sem, 64 * (kf - 1))  # issue throttle
                    for b in range(BL):
                        sync.dma_start(
                            out=blkv(xb_r[kf % 3], b), in_=x_src(b, kf)
                        ).then_inc(x_sem, 16)
                if k >= 1:
                    kk = k - 1
                    sync.wait_ge(spk_sem, TBLK * (kk + 1))
                    for b in range(BL):
                        sync.dma_start(
                            out=out_dst(spikes16, b, kk),
                            in_=blkv(s16_r[kk % 2], b),
                        ).then_inc(so_sem, 16)
                    sync.wait_ge(m16_sem, kk + 1)
                    for b in range(BL):
                        sync.dma_start(
                            out=out_dst(mems16, b, kk),
                            in_=blkv(m16_r[kk % 2], b),
                        ).then_inc(mo_sem, 16)
            kk = NBLK - 1
            sync.wait_ge(spk_sem, TBLK * NBLK)
            for b in range(BL):
                sync.dma_start(
                    out=out_dst(spikes16, b, kk), in_=blkv(s16_r[kk % 2], b)
                ).then_inc(so_sem, 16)
            sync.wait_ge(m16_sem, NBLK)
            for b in range(BL):
                sync.dma_start(
                    out=out_dst(mems16, b, kk), in_=blkv(m16_r[kk % 2], b)
                ).then_inc(mo_sem, 16)
            sync.wait_ge(mo_sem, 64 * NBLK)
            sync.wait_ge(so_sem, 64 * NBLK)

        @block.vector
        def _(vector):
            vector.wait_ge(c_sem, 16 * BL * 2)  # sigth + cm tiles
            for k in range(NBLK):
                if k == 0:
                    vector.wait_ge(x0_sem, 64)  # prologue xb block 0
                else:
                    vector.wait_ge(x_sem, 64 * k)  # xb block k in SBUF
                if k == 0:
                    # per-t xhs with the untiled sigth (no need to wait for
                    # the ACT-side const replication); slice 0 first, the
                    # rest are emitted just-in-time inside the tl loop
                    nc.vector.tensor_tensor(
                        out=xs_t[:, 0:NL],
                        in0=xb_r[0][:, 0:NL],
                        in1=sigth_t[:],
                        op=ALU.mult,
                    )
                else:
                    if k == 1:
                        vector.wait_ge(rep_sem, 2 * TBLK)  # tiled consts ready
                    nc.vector.tensor_tensor(
                        out=xs_t[:],
                        in0=xb_r[k % 3][:],
                        in1=sigthb_t[:],
                        op=ALU.mult,
                    ).then_inc(xsd_sem, 1)
                if k == NBLK - 1:
                    # last block writes its own m16 slot per-t; previous
                    # occupant is block k-2's mems, the (k-1)th mems-DMA
                    vector.wait_ge(mo_sem, 64 * (k - 1))
                if k >= 1:
                    # mems16 for block k-1 from its p tiles, bf16 out.
                    # ring WAR: the slot held block k-3's mems, DMA'd already
                    if k >= 3:
                        vector.wait_ge(mo_sem, 64 * (k - 2))
                    nc.vector.tensor_tensor(
                        out=m16_r[(k - 1) % 2][:],
                        in0=pblk(k - 1),
                        in1=thcmb_t[:],
                        op=ALU.mult,
                    ).then_inc(m16_sem, 1)

                    # w slots: ACT consumed block k-1's w's
                    vector.wait_ge(spk_sem, TBLK * k)
                    # deferred add: uh_{8k} = p_{8k-1} + xhs_{8k}
                    nc.vector.tensor_tensor(
                        out=uhsl(8 * k),
                        in0=psl(k - 1, TBLK - 1),
                        in1=xs_t[:, 0:NL],
                        op=ALU.add,
                    )
                for tl in range(TBLK):
                    t = k * TBLK + tl
                    uh = xs_t[:, 0:NL] if t == 0 else uhsl(t)
                    nc.vector.scalar_tensor_tensor(
                        out=wsl(tl),
                        in0=uh,
                        scalar=1.0,
                        in1=cm_t[:],
                        op0=ALU.is_lt,
                        op1=ALU.mult,
                    ).then_inc(w_sem, 1)
                    nc.vector.tensor_tensor(
                        out=psl(k, tl), in0=uh, in1=wsl(tl), op=ALU.mult
                    )
                    if k == 0 and tl < TBLK - 1:
                        ins0 = nc.vector.tensor_tensor(
                            out=xs_t[:, (tl + 1) * NL : (tl + 2) * NL],
                            in0=xb_r[0][:, (tl + 1) * NL : (tl + 2) * NL],
                            in1=sigth_t[:],
                            op=ALU.mult,
                        )
                        if tl == TBLK - 2:
                            ins0.then_inc(xsd_sem, 1)
                    if k == NBLK - 1:
                        ins = nc.vector.tensor_tensor(
                            out=m16_r[(NBLK - 1) % 2][
                                :, tl * NL : (tl + 1) * NL
                            ],
                            in0=psl(k, tl),
                            in1=thcmb_t[:, tl * NL : (tl + 1) * NL],
                            op=ALU.mult,
                        )
                        if tl == TBLK - 1:
                            ins.then_inc(m16_sem, 1)
                    if tl < TBLK - 1:
                        nc.vector.tensor_tensor(
                            out=uhsl(t + 1),
                            in0=psl(k, tl),
                            in1=xs_t[:, (tl + 1) * NL : (tl + 2) * NL],
                            op=ALU.add,
                        )

        @block.gpsimd
        def _(gp):
            nc.gpsimd.dma_start(out=blkv(xb_r[0], 0), in_=x_src(0, 0)).then_inc(
                x0_sem, 16
            )
            nc.gpsimd.dma_start(out=blkv(xb_r[0], 1), in_=x_src(1, 0)).then_inc(
                x0_sem, 16
            )

        @block.scalar
        def _(scalar):
            nc.scalar.dma_start(out=blkv(xb_r[0], 2), in_=x_src(2, 0)).then_inc(
                x0_sem, 16
            )
            nc.scalar.dma_start(out=blkv(xb_r[0], 3), in_=x_src(3, 0)).then_inc(
                x0_sem, 16
            )
            scalar.wait_ge(c_sem, 16 * BL * 3)
            for tl in range(TBLK):
                nc.scalar.copy(
                    out=sigthb_t[:, tl * NL : (tl + 1) * NL], in_=sigth_t[:]
                ).then_inc(rep_sem, 1)
            for tl in range(TBLK):
                nc.scalar.copy(
                    out=thcmb_t[:, tl * NL : (tl + 1) * NL], in_=thcm_t[:]
                ).then_inc(rep_sem, 1)
            for k in range(NBLK):
                if k >= 2:
                    scalar.wait_ge(so_sem, 64 * (k - 1))  # s16 slot WAR
                for tl in range(TBLK):
                    t = k * TBLK + tl
                    scalar.wait_ge(w_sem, t + 1)
                    # w == 0 exactly iff spike; exp(-1e30*w) = 1.0 at w=0 and
                    # underflows to exactly 0.0 at w = cm >= ~0.1
                    nc.scalar.activation(
                        s16_r[k % 2][:, tl * NL : (tl + 1) * NL],
                        wsl(tl),
                        AF.Exp,
                        scale=-1e30,
                    ).then_inc(spk_sem, 1)

    return nc


def _get_nc() -> bass.Bass:
    if "nc" not in _CACHE:
        _CACHE["nc"] = _build_nc()
    return _CACHE["nc"]


def kernel(x, thresh, tau_x, _trace: bool = False, _tmpdir: str | None = None):
    x = np.ascontiguousarray(np.asarray(x, dtype=np.float32))
    thresh = np.ascontiguousarray(np.asarray(thresh, dtype=np.float32))
    tau_x = np.ascontiguousarray(np.asarray(tau_x, dtype=np.float32))
    assert x.shape == (B, T, N)

    # O(N) host-side constants; all O(B*T*N) math happens on-device.
    sig = (1.0 / (1.0 + np.exp(-tau_x.astype(np.float64)))).astype(np.float32)
    cm = (np.float32(1.0) - sig).astype(np.float32)
    sigth = (sig / thresh).astype(np.float32)
    thcm = (thresh / cm).astype(np.float32)

    nc = _get_nc()
    in_maps = [
        {"x": x[i * BL : (i + 1) * BL], "cm": cm, "sigth": sigth, "thcm": thcm}
        for i in range(NCORES)
    ]
    res = run_bass_kernel_spmd(
        nc, in_maps, core_ids=list(range(NCORES)), trace=_trace, tmpdir=_tmpdir
    )
    spikes = np.concatenate(
        [np.asarray(r["spikes16"]).astype(np.float32) for r in res.results], axis=0
    )
    mems = np.concatenate(
        [np.asarray(r["mems16"]).astype(np.float32) for r in res.results], axis=0
    )
    if _trace:
        _CACHE["last_results"] = res
    return spikes, mems

